# revision 21
# baseline (speedup 1.0000x reference)
"""Channel-attention transformer block on 8 Trainium2 NeuronCores.

Reference semantics (b=8, l=4096, c=512, h=8 heads carved from the
*sequence* axis, head_pos = l % 8):
    qkv = x @ w_qkv.T ; split q,k,v per head  (each (lh=512, c=512))
    attn = softmax((q.T @ k) / 8, axis=-1)    # (c, c) channel attention
    y.T  = attn @ v.T                         # (c, lh)
    out  = y @ w_out.T + b_out

Sharding: data-parallel over batch — core i handles batch i.

Per-core layout trick: the l axis is permuted on the host so each head's
512 rows are contiguous (row h*512+i <- original row i*8+h), and x is
shipped transposed (c, l). Then per head:
  - Q,K in natural (l, c) layout and V^T in (c, l) layout all come
    straight out of matmuls against xT (no on-device transposes),
  - scores are computed *transposed* (S^T = K^T Q via lhsT=K, rhs=Q) so
    softmax's sum over the attended axis lands on the partition dim,
    where it is computed by a matmul against ones columns glued onto
    V^T (columns 0-1 of the AV rhs) — again no transposes,
  - normalization (multiply by 1/denominator, a per-partition scalar)
    is fused into the PSUM->SBUF evacuation of the AV result,
  - the out-projection consumes y^T (c on partitions) directly as lhsT.
The host un-permutes rows of the returned (4096, 512) per-core output.
"""

import numpy as np

import concourse.bass as bass
import concourse.mybir as mybir
import concourse.tile as tile
from concourse.bass_utils import run_bass_kernel_spmd

B = 8
L = 4096
C = 512
HEADS = 8
LH = L // HEADS  # 512
SCALE = 64 ** -0.5  # DIM_HEAD ** -0.5 from the reference
N_CORES = 8
P = 128
KC = C // P  # 4 contraction chunks of 128
F32 = mybir.dt.float32

# Matmul operand dtype: bfloat16. Streams at the same 1 col/cycle as
# float32r, but (a) enables the compiler's Fast Weight Load for LDWEIGHTS
# (4-byte reads; fp32r weights load one element/cycle and leave the PE
# waiting on weight loads), (b) halves input DMA bytes and SBUF footprint.
# Measured end-to-end error vs the fp32 reference is ~8.5e-3 (tolerance
# 2e-2); accumulation stays fp32 in PSUM.
MM_DTYPE = mybir.dt.bfloat16
PD = MM_DTYPE  # dtype of every tile that feeds a matmul

# Dummy matmuls issued right after the ~7us engine preamble, before any
# input DMA lands. The PE clock is HAM-gated to 1.2 GHz until the PE has
# been *continuously* busy ~4us; any idle hole resets the streak. The
# spins must therefore bridge, without a gap, from t~8us through the
# ungate (~12.5us) to when the first projection inputs have landed
# spins bridge the gap; the real stream then keeps the streak alive.
# (~13us). With contiguous input DMAs the blocks land by ~10us, so 9
WARMUP_MM = 20


def _split_wide_waits(nc, max_waits=1):
    """This container's walrus build rejects instructions carrying more than
    ~1 sync wait ("Too many sync wait commands", e.g. in the S3_LW lowering
    of a fused matmul). Hoist surplus waits onto same-engine nops inserted
    immediately before the offending instruction — the engine stalls at the
    same point in its stream, so scheduling semantics are unchanged."""
    for f in nc.m.functions:
        for bb in f.blocks:
            snapshot = list(bb.instructions)
            if not any(
                inst.sync_info and inst.sync_info.on_wait
                and len(inst.sync_info.on_wait) > max_waits
                for inst in snapshot
            ):
                continue
            new = []
            for inst in snapshot:
                si = inst.sync_info
                waits = list(si.on_wait) if si and si.on_wait else []
                if len(waits) > max_waits:
                    for w in waits[:-max_waits]:
                        nop = nc.engines[inst.engine].nop(nofuse=True).ins
                        cur = nc.cur_bb.bb.instructions
                        assert cur[-1] is nop
                        cur.pop()  # re-homed below, right before `inst`
                        nop.sync_info = mybir.SyncInfo(on_wait=[w], on_update=[])
                        new.append(nop)
                    inst.sync_info = mybir.SyncInfo(
                        on_wait=waits[-max_waits:],
                        on_update=list(si.on_update) if si.on_update else [],
                    )
                new.append(inst)
            bb.instructions = new


def _emit(ctx, tc, xh, wqh, wkh, wvh, woh, out):
    """Emit the per-core program. DRAM inputs are bf16, pre-permuted on the
    host so every DMA is contiguous per partition (4KB segments):
    xh (HEADS, P, KC, LH), wqh/wkh/wvh/woh (P, KC, C) with the attention
    scale folded into wqh. out (L, C) fp32."""
    nc = tc.nc
    EXP = mybir.ActivationFunctionType.Exp

    out_r = out.rearrange("(s p) c -> p s c", p=P)

    consts = ctx.enter_context(tc.tile_pool(name="consts", bufs=1))
    xt_pool = ctx.enter_context(tc.tile_pool(name="xt", bufs=3))
    q_pool = ctx.enter_context(tc.tile_pool(name="q", bufs=3))
    k_pool = ctx.enter_context(tc.tile_pool(name="k", bufs=3))
    vt_pool = ctx.enter_context(tc.tile_pool(name="vt", bufs=3))
    exp_pool = ctx.enter_context(tc.tile_pool(name="exp", bufs=3))
    y_pool = ctx.enter_context(tc.tile_pool(name="y", bufs=3))
    out_pool = ctx.enter_context(tc.tile_pool(name="out", bufs=8))
    recip_pool = ctx.enter_context(tc.tile_pool(name="recip", bufs=8))
    # PSUM: 8 banks total. 4 cycle through proj/scores/AV matmul groups;
    # 4 are held by the out-projection's ko-accumulation (one per l' strip).
    pp_mm = ctx.enter_context(tc.tile_pool(name="pp_mm", bufs=4, space="PSUM"))
    pp_out = ctx.enter_context(tc.tile_pool(name="pp_out", bufs=4, space="PSUM"))

    # PE warmup spin: keep the tensor engine busy from t~0 (no DMA deps)
    # so the HAM clock gate releases to 2.4 GHz before real work arrives.
    warm = consts.tile([P, 2 * P], PD)
    nc.vector.memset(warm[:], 1.0)
    wpsum = pp_mm.tile([P, 2 * P], F32, tag="mm")
    for _ in range(WARMUP_MM):
        nc.tensor.matmul(wpsum[:], warm[:, 0:P], warm[:], start=True, stop=True)

    # Startup critical path: nothing can transfer until the ~7us engine
    # preamble finishes, so spread the first loads over all three DMA
    # rails (sync + scalar HWDGE, gpsimd SWDGE): q-weights on sync,
    # k-weights on scalar, head-0 x block on gpsimd; then v-weights /
    # out-weights follow on the two HWDGE rails.
    wq = consts.tile([P, KC, C], PD)
    wk = consts.tile([P, KC, C], PD)
    wv = consts.tile([P, KC, C], PD)
    wout = consts.tile([P, KC, C], PD)
    xth0 = xt_pool.tile([P, KC, LH], PD, tag="xth")
    # Each block is split into partition-quarter chunks: a single dma_start
    # is serviced by ONE DMA engine (~40GB/s), so 4 chunks per queue run 4
    # engines in parallel, and the contiguous host layout keeps segments
    # large. PQ quarters of 32 partitions each.
    PQ = P // 4
    for i in range(4):
        s = bass.ds(i * PQ, PQ)
        nc.gpsimd.dma_start(xth0[s], xh[0, s])
        nc.sync.dma_start(wq[s], wqh[s])
        nc.scalar.dma_start(wk[s], wkh[s])
    for i in range(4):
        s = bass.ds(i * PQ, PQ)
        nc.sync.dma_start(wv[s], wvh[s])
        nc.scalar.dma_start(wout[s], woh[s])

    for h in range(HEADS):
        if h == 0:
            xth = xth0
        else:
            xth = xt_pool.tile([P, KC, LH], PD, tag="xth")
            for i in range(4):
                s = bass.ds(i * (P // 4), P // 4)
                nc.gpsimd.dma_start(xth[s], xh[h, s])

        # ---- projections Q,K natural (l, c); q evacuates via the scalar
        # engine, k via vector, so neither engine backlogs the scores ----
        q = q_pool.tile([P, KC, C], PD)
        k = k_pool.tile([P, KC, C], PD)
        COPY_FN = mybir.ActivationFunctionType.Copy
        for m in range(KC):  # l' strips of 128
            for w_j, dst in ((wq, q), (wk, k)):
                pq = pp_mm.tile([P, C], F32, tag="mm")
                for ko in range(KC):
                    nc.tensor.matmul(
                        pq[:], xth[:, ko, bass.ts(m, P)],
                        w_j[:, ko, :],
                        start=(ko == 0), stop=(ko == KC - 1))
                if dst is q:
                    nc.scalar.activation(dst[:, m, :], pq[:], COPY_FN)
                else:
                    nc.vector.tensor_copy(dst[:, m, :], pq[:])

        # ---- V^T projection sits between the q/k projections and scores:
        # its operands are ready early, and it gives the PE ~3.4us of work
        # while the last q/k strips evacuate ahead of the scores stage ----
        vt = vt_pool.tile([P, KC, LH + 2], PD)
        nc.vector.memset(vt[:, :, 0:2], 1.0)
        for m in range(KC):  # c_v strips of 128
            pv = pp_mm.tile([P, LH], F32, tag="mm")
            for ko in range(KC):
                nc.tensor.matmul(
                    pv[:], wv[:, ko, bass.ds(m * P, P)],
                    xth[:, ko, :],
                    start=(ko == 0), stop=(ko == KC - 1))
            nc.vector.tensor_copy(vt[:, m, 2:LH + 2], pv[:])

        # ---- scores transposed + exp:  S^T[d, c] = sum_l K[l,d] Q[l,c] ----
        ex = exp_pool.tile([P, KC, C], PD)
        for ds_ in range(KC):  # d strips of 128
            ps = pp_mm.tile([P, C], F32, tag="mm")
            for m in range(KC):  # contraction over l' chunks
                nc.tensor.matmul(
                    ps[:], k[:, m, bass.ts(ds_, P)],
                    q[:, m, :],
                    start=(m == 0), stop=(m == KC - 1))
            nc.scalar.activation(ex[:, ds_, :], ps[:], EXP)

        # ---- AV with fused denominator (rhs cols 0,1 are ones; the even
        # N-split respects the 512-fp32 PSUM bank limit). Normalize is a
        # scalar-engine Copy with per-partition scale 1/denominator. ----
        NY1 = 258  # 2 (denominator twice) + 256 v columns
        NY2 = 256
        y = y_pool.tile([P, KC, LH], PD)
        for cs in range(KC):  # c strips of 128
            py1 = pp_mm.tile([P, NY1], F32, tag="mm")
            py2 = pp_mm.tile([P, NY2], F32, tag="mm")
            for ko in range(KC):  # contraction over d chunks
                lhsT = ex[:, ko, bass.ts(cs, P)]
                nc.tensor.matmul(py1[:], lhsT, vt[:, ko, 0:NY1],
                                 start=(ko == 0), stop=(ko == KC - 1))
            for ko in range(KC):
                lhsT = ex[:, ko, bass.ts(cs, P)]
                nc.tensor.matmul(py2[:], lhsT, vt[:, ko, NY1:LH + 2],
                                 start=(ko == 0), stop=(ko == KC - 1))
            rc = recip_pool.tile([P, 1], F32)
            nc.vector.reciprocal(rc[:], py1[:, 0:1])
            nc.scalar.activation(y[:, cs, 0:NY1 - 2], py1[:, 2:NY1], COPY_FN,
                                 scale=rc[:])
            nc.scalar.activation(y[:, cs, NY1 - 2:LH], py2[:], COPY_FN,
                                 scale=rc[:])

        # ---- out projection: out[l, co] = sum_c y^T[c, l] woutT[c, co].
        # ko-major accumulation into 4 held PSUM banks: chunk ko only needs
        # y strip ko, so these matmuls chase the AV normalizes instead of
        # waiting for the whole y tile. ----
        po = [pp_out.tile([P, C], F32, name=f"po{m}", tag="po")
              for m in range(KC)]
        for ko in range(KC):
            for m in range(KC):
                nc.tensor.matmul(
                    po[m][:], y[:, ko, bass.ts(m, P)],
                    wout[:, ko, :],
                    start=(ko == 0), stop=(ko == KC - 1))
        for t in range(KC // 2):  # pairs of l' strips -> one DMA each
            ot = out_pool.tile([P, 2, C], F32)
            nc.vector.tensor_copy(ot[:, 0, :], po[2 * t][:])
            nc.scalar.activation(ot[:, 1, :], po[2 * t + 1][:], COPY_FN)
            dst = out_r[:, bass.ds(h * KC + 2 * t, 2), :]
            if t == 0:
                nc.sync.dma_start(dst, ot[:])
            else:
                nc.gpsimd.dma_start(dst, ot[:])


def _build_program():
    nc = bass.Bass(trn_type="TRN2", target_bir_lowering=False, debug=False,
                   num_devices=N_CORES)
    xh = nc.dram_tensor("xh", [HEADS, P, KC, LH], PD, kind="ExternalInput").ap()
    wqh = nc.dram_tensor("wqh", [P, KC, C], PD, kind="ExternalInput").ap()
    wkh = nc.dram_tensor("wkh", [P, KC, C], PD, kind="ExternalInput").ap()
    wvh = nc.dram_tensor("wvh", [P, KC, C], PD, kind="ExternalInput").ap()
    woh = nc.dram_tensor("woh", [P, KC, C], PD, kind="ExternalInput").ap()
    out = nc.dram_tensor("out", [L, C], F32, kind="ExternalOutput").ap()

    from contextlib import ExitStack
    with tile.TileContext(nc) as tc:
        with ExitStack() as ctx:
            _emit(ctx, tc, xh, wqh, wkh, wvh, woh, out)
    _split_wide_waits(nc)
    return nc


def _w_host(w_t):
    """(C, N) transposed weight -> (P, KC, N): row c = ko*P + p goes to
    [p, ko, :], contiguous per partition so the DMA uses large segments."""
    n = w_t.shape[1]
    return np.ascontiguousarray(w_t.reshape(KC, P, n).transpose(1, 0, 2))


def _host_inputs(x, w_qkv, w_out):
    """Per-core input maps, all bf16, pre-permuted so on-device DMAs are
    contiguous per partition. l is permuted so head h owns rows
    [h*512, (h+1)*512) (original row i*8+h -> permuted row h*512+i), and
    x ships transposed (c on partitions)."""
    import ml_dtypes
    BF16 = ml_dtypes.bfloat16
    wqkv_t = np.ascontiguousarray(w_qkv.T).astype(np.float32).copy()
    wqkv_t[:, 0:C] *= SCALE  # fold the attention scale into the Q weights
    wqkv_t = wqkv_t.astype(BF16)
    wqh = _w_host(wqkv_t[:, 0:C])
    wkh = _w_host(wqkv_t[:, C:2 * C])
    wvh = _w_host(wqkv_t[:, 2 * C:])
    woh = _w_host(np.ascontiguousarray(w_out.T).astype(BF16))
    in_maps = []
    for b in range(B):
        xb = x[b]  # (L, C); row l = i*8 + h
        x_perm = xb.reshape(LH, HEADS, C).transpose(1, 0, 2).reshape(L, C)
        xt = np.ascontiguousarray(x_perm.T).astype(BF16)  # (C, L)
        # (HEADS, P, KC, LH): xh[h, p, ko, l] = xt[ko*P + p, h*LH + l]
        xh = np.ascontiguousarray(
            xt.reshape(KC, P, HEADS, LH).transpose(2, 1, 0, 3))
        in_maps.append({"xh": xh, "wqh": wqh, "wkh": wkh, "wvh": wvh,
                        "woh": woh})
    return in_maps


def _unpermute(out_perm):
    """(L, C) with rows grouped by head -> original row order i*8+h."""
    return out_perm.reshape(HEADS, LH, C).transpose(1, 0, 2).reshape(L, C)


def kernel(x, w_qkv, w_out, b_out, _run_kwargs=None):
    x = np.asarray(x, dtype=np.float32)
    w_qkv = np.asarray(w_qkv, dtype=np.float32)
    w_out = np.asarray(w_out, dtype=np.float32)
    b_out = np.asarray(b_out, dtype=np.float32)

    nc = _build_program()
    in_maps = _host_inputs(x, w_qkv, w_out)
    res = run_bass_kernel_spmd(nc, in_maps, list(range(N_CORES)),
                               **(_run_kwargs or {}))
    out = np.empty((B, L, C), dtype=np.float32)
    for b in range(B):
        out[b] = _unpermute(res.results[b]["out"])
    out += b_out
    if _run_kwargs:
        kernel.last_result = res
    return out



# revision 22
# speedup vs baseline: 1.0292x; 1.0292x over previous
"""Channel-attention transformer block on 8 Trainium2 NeuronCores.

Reference semantics (b=8, l=4096, c=512, h=8 heads carved from the
*sequence* axis, head_pos = l % 8):
    qkv = x @ w_qkv.T ; split q,k,v per head  (each (lh=512, c=512))
    attn = softmax((q.T @ k) / 8, axis=-1)    # (c, c) channel attention
    y.T  = attn @ v.T                         # (c, lh)
    out  = y @ w_out.T + b_out

Sharding: data-parallel over batch — core i handles batch i.

Per-core layout trick: the l axis is permuted on the host so each head's
512 rows are contiguous (row h*512+i <- original row i*8+h), and x is
shipped transposed (c, l). Then per head:
  - Q,K in natural (l, c) layout and V^T in (c, l) layout all come
    straight out of matmuls against xT (no on-device transposes),
  - scores are computed *transposed* (S^T = K^T Q via lhsT=K, rhs=Q) so
    softmax's sum over the attended axis lands on the partition dim,
    where it is computed by a matmul against ones columns glued onto
    V^T (columns 0-1 of the AV rhs) — again no transposes,
  - normalization (multiply by 1/denominator, a per-partition scalar)
    is fused into the PSUM->SBUF evacuation of the AV result,
  - the out-projection consumes y^T (c on partitions) directly as lhsT.
The host un-permutes rows of the returned (4096, 512) per-core output.
"""

import numpy as np

import concourse.bass as bass
import concourse.mybir as mybir
import concourse.tile as tile
from concourse.bass_utils import run_bass_kernel_spmd

B = 8
L = 4096
C = 512
HEADS = 8
LH = L // HEADS  # 512
SCALE = 64 ** -0.5  # DIM_HEAD ** -0.5 from the reference
N_CORES = 8
P = 128
KC = C // P  # 4 contraction chunks of 128
F32 = mybir.dt.float32

# Matmul operand dtype: bfloat16. Streams at the same 1 col/cycle as
# float32r, but (a) enables the compiler's Fast Weight Load for LDWEIGHTS
# (4-byte reads; fp32r weights load one element/cycle and leave the PE
# waiting on weight loads), (b) halves input DMA bytes and SBUF footprint.
# Measured end-to-end error vs the fp32 reference is ~8.5e-3 (tolerance
# 2e-2); accumulation stays fp32 in PSUM.
MM_DTYPE = mybir.dt.bfloat16
PD = MM_DTYPE  # dtype of every tile that feeds a matmul

# Dummy matmuls issued right after the ~7us engine preamble, before any
# input DMA lands. The PE clock is HAM-gated to 1.2 GHz until the PE has
# been *continuously* busy ~4us; any idle hole resets the streak. The
# spins must therefore bridge, without a gap, from t~8us through the
# ungate (~12.5us) to when the first projection inputs have landed
# spins bridge the gap; the real stream then keeps the streak alive.
# (~13us). With contiguous input DMAs the blocks land by ~10us, so 9
WARMUP_MM = 20


def _split_wide_waits(nc, max_waits=1):
    """This container's walrus build rejects instructions carrying more than
    ~1 sync wait ("Too many sync wait commands", e.g. in the S3_LW lowering
    of a fused matmul). Hoist surplus waits onto same-engine nops inserted
    immediately before the offending instruction — the engine stalls at the
    same point in its stream, so scheduling semantics are unchanged."""
    for f in nc.m.functions:
        for bb in f.blocks:
            snapshot = list(bb.instructions)
            if not any(
                inst.sync_info and inst.sync_info.on_wait
                and len(inst.sync_info.on_wait) > max_waits
                for inst in snapshot
            ):
                continue
            new = []
            for inst in snapshot:
                si = inst.sync_info
                waits = list(si.on_wait) if si and si.on_wait else []
                if len(waits) > max_waits:
                    for w in waits[:-max_waits]:
                        nop = nc.engines[inst.engine].nop(nofuse=True).ins
                        cur = nc.cur_bb.bb.instructions
                        assert cur[-1] is nop
                        cur.pop()  # re-homed below, right before `inst`
                        nop.sync_info = mybir.SyncInfo(on_wait=[w], on_update=[])
                        new.append(nop)
                    inst.sync_info = mybir.SyncInfo(
                        on_wait=waits[-max_waits:],
                        on_update=list(si.on_update) if si.on_update else [],
                    )
                new.append(inst)
            bb.instructions = new


def _emit(ctx, tc, xh, wqh, wkh, wvh, woh, out):
    """Emit the per-core program. DRAM inputs are bf16, pre-permuted on the
    host so every DMA is contiguous per partition (4KB segments):
    xh (HEADS, P, KC, LH), wqh/wkh/wvh/woh (P, KC, C) with the attention
    scale folded into wqh. out (L, C) fp32."""
    nc = tc.nc
    EXP = mybir.ActivationFunctionType.Exp

    out_r = out.rearrange("(s p) c -> p s c", p=P)

    consts = ctx.enter_context(tc.tile_pool(name="consts", bufs=1))
    xt_pool = ctx.enter_context(tc.tile_pool(name="xt", bufs=3))
    q_pool = ctx.enter_context(tc.tile_pool(name="q", bufs=3))
    k_pool = ctx.enter_context(tc.tile_pool(name="k", bufs=3))
    vt_pool = ctx.enter_context(tc.tile_pool(name="vt", bufs=3))
    exp_pool = ctx.enter_context(tc.tile_pool(name="exp", bufs=3))
    y_pool = ctx.enter_context(tc.tile_pool(name="y", bufs=3))
    out_pool = ctx.enter_context(tc.tile_pool(name="out", bufs=8))
    recip_pool = ctx.enter_context(tc.tile_pool(name="recip", bufs=8))
    # PSUM: all 8 banks cycle through one pool so the AV stage can keep
    # all four strips' tiles in flight ahead of their evacuations.
    pp_mm = ctx.enter_context(tc.tile_pool(name="pp_mm", bufs=8, space="PSUM"))

    # PE warmup spin: keep the tensor engine busy from t~0 (no DMA deps)
    # so the HAM clock gate releases to 2.4 GHz before real work arrives.
    warm = consts.tile([P, 2 * P], PD)
    nc.vector.memset(warm[:], 1.0)
    wpsum = pp_mm.tile([P, 2 * P], F32, tag="mm")
    for _ in range(WARMUP_MM):
        nc.tensor.matmul(wpsum[:], warm[:, 0:P], warm[:], start=True, stop=True)

    # Startup critical path: nothing can transfer until the ~7us engine
    # preamble finishes, so spread the first loads over all three DMA
    # rails (sync + scalar HWDGE, gpsimd SWDGE): q-weights on sync,
    # k-weights on scalar, head-0 x block on gpsimd; then v-weights /
    # out-weights follow on the two HWDGE rails.
    wq = consts.tile([P, KC, C], PD)
    wk = consts.tile([P, KC, C], PD)
    wv = consts.tile([P, KC, C], PD)
    wout = consts.tile([P, KC, C], PD)
    xth0 = xt_pool.tile([P, KC, LH], PD, tag="xth")
    # Each block is split into partition-quarter chunks: a single dma_start
    # is serviced by ONE DMA engine (~40GB/s), so 4 chunks per queue run 4
    # engines in parallel, and the contiguous host layout keeps segments
    # large. PQ quarters of 32 partitions each.
    PQ = P // 4
    for i in range(4):
        s = bass.ds(i * PQ, PQ)
        nc.gpsimd.dma_start(xth0[s], xh[0, s])
        nc.sync.dma_start(wq[s], wqh[s])
        nc.scalar.dma_start(wk[s], wkh[s])
    for i in range(4):
        s = bass.ds(i * PQ, PQ)
        nc.sync.dma_start(wv[s], wvh[s])
        nc.scalar.dma_start(wout[s], woh[s])

    for h in range(HEADS):
        if h == 0:
            xth = xth0
        else:
            xth = xt_pool.tile([P, KC, LH], PD, tag="xth")
            # single-engine background transfer; the gpsimd rail carries
            # ONLY x prefetches so this trigger is never queued behind a
            # dependent DMA and fires as soon as the buffer frees
            nc.gpsimd.dma_start(xth[s], xh[h, s]) if False else \
                nc.gpsimd.dma_start(xth[:], xh[h])

        # ---- projections Q,K natural (l, c); q evacuates via the scalar
        # engine, k via vector, so neither engine backlogs the scores ----
        q = q_pool.tile([P, KC, C], PD)
        k = k_pool.tile([P, KC, C], PD)
        COPY_FN = mybir.ActivationFunctionType.Copy
        for m in range(KC):  # l' strips of 128
            for w_j, dst in ((wq, q), (wk, k)):
                pq = pp_mm.tile([P, C], F32, tag="mm")
                for ko in range(KC):
                    nc.tensor.matmul(
                        pq[:], xth[:, ko, bass.ts(m, P)],
                        w_j[:, ko, :],
                        start=(ko == 0), stop=(ko == KC - 1))
                if dst is q:
                    nc.scalar.activation(dst[:, m, :], pq[:], COPY_FN)
                else:
                    nc.vector.tensor_copy(dst[:, m, :], pq[:])

        # ---- V^T projection sits between the q/k projections and scores:
        # its operands are ready early, and it gives the PE ~3.4us of work
        # while the last q/k strips evacuate ahead of the scores stage ----
        vt = vt_pool.tile([P, KC, LH + 2], PD)
        nc.vector.memset(vt[:, :, 0:2], 1.0)
        for m in range(KC):  # c_v strips of 128
            pv = pp_mm.tile([P, LH], F32, tag="mm")
            for ko in range(KC):
                nc.tensor.matmul(
                    pv[:], wv[:, ko, bass.ds(m * P, P)],
                    xth[:, ko, :],
                    start=(ko == 0), stop=(ko == KC - 1))
            nc.vector.tensor_copy(vt[:, m, 2:LH + 2], pv[:])

        # ---- scores transposed + exp:  S^T[d, c] = sum_l K[l,d] Q[l,c] ----
        ex = exp_pool.tile([P, KC, C], PD)
        for ds_ in range(KC):  # d strips of 128
            ps = pp_mm.tile([P, C], F32, tag="mm")
            for m in range(KC):  # contraction over l' chunks
                nc.tensor.matmul(
                    ps[:], k[:, m, bass.ts(ds_, P)],
                    q[:, m, :],
                    start=(m == 0), stop=(m == KC - 1))
            nc.scalar.activation(ex[:, ds_, :], ps[:], EXP)

        # ---- AV with fused denominator (rhs cols 0,1 are ones; the even
        # N-split respects the 512-fp32 PSUM bank limit). Normalize is a
        # scalar-engine Copy with per-partition scale 1/denominator. ----
        NY1 = 258  # 2 (denominator twice) + 256 v columns
        NY2 = 256
        y = y_pool.tile([P, KC, LH], PD)
        for cs in range(KC):  # c strips of 128
            py1 = pp_mm.tile([P, NY1], F32, tag="mm")
            py2 = pp_mm.tile([P, NY2], F32, tag="mm")
            for ko in range(KC):  # contraction over d chunks
                lhsT = ex[:, ko, bass.ts(cs, P)]
                nc.tensor.matmul(py1[:], lhsT, vt[:, ko, 0:NY1],
                                 start=(ko == 0), stop=(ko == KC - 1))
            for ko in range(KC):
                lhsT = ex[:, ko, bass.ts(cs, P)]
                nc.tensor.matmul(py2[:], lhsT, vt[:, ko, NY1:LH + 2],
                                 start=(ko == 0), stop=(ko == KC - 1))
            rc = recip_pool.tile([P, 1], F32)
            nc.vector.reciprocal(rc[:], py1[:, 0:1])
            nc.scalar.activation(y[:, cs, 0:NY1 - 2], py1[:, 2:NY1], COPY_FN,
                                 scale=rc[:])
            nc.vector.tensor_scalar_mul(y[:, cs, NY1 - 2:LH], py2[:], rc[:])

        # ---- out projection: out[l, co] = sum_c y^T[c, l] woutT[c, co];
        # strip copies alternate vector/scalar, strip-pair DMAs alternate
        # sync/scalar so the gpsimd rail stays x-only ----
        for t in range(KC // 2):  # pairs of l' strips -> one DMA each
            ot = out_pool.tile([P, 2, C], F32)
            for u in range(2):
                m = 2 * t + u
                po = pp_mm.tile([P, C], F32, tag="mm")
                for ko in range(KC):
                    nc.tensor.matmul(
                        po[:], y[:, ko, bass.ts(m, P)],
                        wout[:, ko, :],
                        start=(ko == 0), stop=(ko == KC - 1))
                if u == 0:
                    nc.vector.tensor_copy(ot[:, u, :], po[:])
                else:
                    nc.scalar.activation(ot[:, u, :], po[:], COPY_FN)
            dst = out_r[:, bass.ds(h * KC + 2 * t, 2), :]
            if t == 0:
                nc.sync.dma_start(dst, ot[:])
            else:
                nc.scalar.dma_start(dst, ot[:])


def _build_program():
    nc = bass.Bass(trn_type="TRN2", target_bir_lowering=False, debug=False,
                   num_devices=N_CORES)
    xh = nc.dram_tensor("xh", [HEADS, P, KC, LH], PD, kind="ExternalInput").ap()
    wqh = nc.dram_tensor("wqh", [P, KC, C], PD, kind="ExternalInput").ap()
    wkh = nc.dram_tensor("wkh", [P, KC, C], PD, kind="ExternalInput").ap()
    wvh = nc.dram_tensor("wvh", [P, KC, C], PD, kind="ExternalInput").ap()
    woh = nc.dram_tensor("woh", [P, KC, C], PD, kind="ExternalInput").ap()
    out = nc.dram_tensor("out", [L, C], F32, kind="ExternalOutput").ap()

    from contextlib import ExitStack
    with tile.TileContext(nc) as tc:
        with ExitStack() as ctx:
            _emit(ctx, tc, xh, wqh, wkh, wvh, woh, out)
    _split_wide_waits(nc)
    return nc


def _w_host(w_t):
    """(C, N) transposed weight -> (P, KC, N): row c = ko*P + p goes to
    [p, ko, :], contiguous per partition so the DMA uses large segments."""
    n = w_t.shape[1]
    return np.ascontiguousarray(w_t.reshape(KC, P, n).transpose(1, 0, 2))


def _host_inputs(x, w_qkv, w_out):
    """Per-core input maps, all bf16, pre-permuted so on-device DMAs are
    contiguous per partition. l is permuted so head h owns rows
    [h*512, (h+1)*512) (original row i*8+h -> permuted row h*512+i), and
    x ships transposed (c on partitions)."""
    import ml_dtypes
    BF16 = ml_dtypes.bfloat16
    wqkv_t = np.ascontiguousarray(w_qkv.T).astype(np.float32).copy()
    wqkv_t[:, 0:C] *= SCALE  # fold the attention scale into the Q weights
    wqkv_t = wqkv_t.astype(BF16)
    wqh = _w_host(wqkv_t[:, 0:C])
    wkh = _w_host(wqkv_t[:, C:2 * C])
    wvh = _w_host(wqkv_t[:, 2 * C:])
    woh = _w_host(np.ascontiguousarray(w_out.T).astype(BF16))
    in_maps = []
    for b in range(B):
        xb = x[b]  # (L, C); row l = i*8 + h
        x_perm = xb.reshape(LH, HEADS, C).transpose(1, 0, 2).reshape(L, C)
        xt = np.ascontiguousarray(x_perm.T).astype(BF16)  # (C, L)
        # (HEADS, P, KC, LH): xh[h, p, ko, l] = xt[ko*P + p, h*LH + l]
        xh = np.ascontiguousarray(
            xt.reshape(KC, P, HEADS, LH).transpose(2, 1, 0, 3))
        in_maps.append({"xh": xh, "wqh": wqh, "wkh": wkh, "wvh": wvh,
                        "woh": woh})
    return in_maps


def _unpermute(out_perm):
    """(L, C) with rows grouped by head -> original row order i*8+h."""
    return out_perm.reshape(HEADS, LH, C).transpose(1, 0, 2).reshape(L, C)


def kernel(x, w_qkv, w_out, b_out, _run_kwargs=None):
    x = np.asarray(x, dtype=np.float32)
    w_qkv = np.asarray(w_qkv, dtype=np.float32)
    w_out = np.asarray(w_out, dtype=np.float32)
    b_out = np.asarray(b_out, dtype=np.float32)

    nc = _build_program()
    in_maps = _host_inputs(x, w_qkv, w_out)
    res = run_bass_kernel_spmd(nc, in_maps, list(range(N_CORES)),
                               **(_run_kwargs or {}))
    out = np.empty((B, L, C), dtype=np.float32)
    for b in range(B):
        out[b] = _unpermute(res.results[b]["out"])
    out += b_out
    if _run_kwargs:
        kernel.last_result = res
    return out



# revision 24
# speedup vs baseline: 1.0553x; 1.0253x over previous
"""Channel-attention transformer block on 8 Trainium2 NeuronCores.

Reference semantics (b=8, l=4096, c=512, h=8 heads carved from the
*sequence* axis, head_pos = l % 8):
    qkv = x @ w_qkv.T ; split q,k,v per head  (each (lh=512, c=512))
    attn = softmax((q.T @ k) / 8, axis=-1)    # (c, c) channel attention
    y.T  = attn @ v.T                         # (c, lh)
    out  = y @ w_out.T + b_out

Sharding: data-parallel over batch — core i handles batch i.

Per-core layout trick: the l axis is permuted on the host so each head's
512 rows are contiguous (row h*512+i <- original row i*8+h), and x is
shipped transposed (c, l). Then per head:
  - Q,K in natural (l, c) layout and V^T in (c, l) layout all come
    straight out of matmuls against xT (no on-device transposes),
  - scores are computed *transposed* (S^T = K^T Q via lhsT=K, rhs=Q) so
    softmax's sum over the attended axis lands on the partition dim,
    where it is computed by a matmul against ones columns glued onto
    V^T (columns 0-1 of the AV rhs) — again no transposes,
  - normalization (multiply by 1/denominator, a per-partition scalar)
    is fused into the PSUM->SBUF evacuation of the AV result,
  - the out-projection consumes y^T (c on partitions) directly as lhsT.
The host un-permutes rows of the returned (4096, 512) per-core output.
"""

import numpy as np

import concourse.bass as bass
import concourse.mybir as mybir
import concourse.tile as tile
from concourse.bass_utils import run_bass_kernel_spmd

B = 8
L = 4096
C = 512
HEADS = 8
LH = L // HEADS  # 512
SCALE = 64 ** -0.5  # DIM_HEAD ** -0.5 from the reference
N_CORES = 8
P = 128
KC = C // P  # 4 contraction chunks of 128
F32 = mybir.dt.float32

# Matmul operand dtype: bfloat16. Streams at the same 1 col/cycle as
# float32r, but (a) enables the compiler's Fast Weight Load for LDWEIGHTS
# (4-byte reads; fp32r weights load one element/cycle and leave the PE
# waiting on weight loads), (b) halves input DMA bytes and SBUF footprint.
# Measured end-to-end error vs the fp32 reference is ~8.5e-3 (tolerance
# 2e-2); accumulation stays fp32 in PSUM.
MM_DTYPE = mybir.dt.bfloat16
PD = MM_DTYPE  # dtype of every tile that feeds a matmul

# Dummy matmuls issued right after the ~7us engine preamble, before any
# input DMA lands. The PE clock is HAM-gated to 1.2 GHz until the PE has
# been *continuously* busy ~4us; any idle hole resets the streak. The
# spins must therefore bridge, without a gap, from t~8us through the
# ungate (~12.5us) to when the first projection inputs have landed
# spins bridge the gap; the real stream then keeps the streak alive.
# (~13us). With contiguous input DMAs the blocks land by ~10us, so 9
WARMUP_MM = 20


def _split_wide_waits(nc, max_waits=1):
    """This container's walrus build rejects instructions carrying more than
    ~1 sync wait ("Too many sync wait commands", e.g. in the S3_LW lowering
    of a fused matmul). Hoist surplus waits onto same-engine nops inserted
    immediately before the offending instruction — the engine stalls at the
    same point in its stream, so scheduling semantics are unchanged."""
    for f in nc.m.functions:
        for bb in f.blocks:
            snapshot = list(bb.instructions)
            if not any(
                inst.sync_info and inst.sync_info.on_wait
                and len(inst.sync_info.on_wait) > max_waits
                for inst in snapshot
            ):
                continue
            new = []
            for inst in snapshot:
                si = inst.sync_info
                waits = list(si.on_wait) if si and si.on_wait else []
                if len(waits) > max_waits:
                    for w in waits[:-max_waits]:
                        nop = nc.engines[inst.engine].nop(nofuse=True).ins
                        cur = nc.cur_bb.bb.instructions
                        assert cur[-1] is nop
                        cur.pop()  # re-homed below, right before `inst`
                        nop.sync_info = mybir.SyncInfo(on_wait=[w], on_update=[])
                        new.append(nop)
                    inst.sync_info = mybir.SyncInfo(
                        on_wait=waits[-max_waits:],
                        on_update=list(si.on_update) if si.on_update else [],
                    )
                new.append(inst)
            bb.instructions = new


def _emit(ctx, tc, xh, wqh, wkh, wvh, woh, out):
    """Emit the per-core program. DRAM inputs are bf16, pre-permuted on the
    host so every DMA is contiguous per partition (4KB segments):
    xh (HEADS, P, KC, LH), wqh/wkh/wvh/woh (P, KC, C) with the attention
    scale folded into wqh. out (L, C) fp32."""
    nc = tc.nc
    EXP = mybir.ActivationFunctionType.Exp

    out_r = out.rearrange("(s p) c -> p s c", p=P)

    consts = ctx.enter_context(tc.tile_pool(name="consts", bufs=1))
    xt_pool = ctx.enter_context(tc.tile_pool(name="xt", bufs=3))
    q_pool = ctx.enter_context(tc.tile_pool(name="q", bufs=3))
    k_pool = ctx.enter_context(tc.tile_pool(name="k", bufs=3))
    vt_pool = ctx.enter_context(tc.tile_pool(name="vt", bufs=3))
    exp_pool = ctx.enter_context(tc.tile_pool(name="exp", bufs=3))
    y_pool = ctx.enter_context(tc.tile_pool(name="y", bufs=3))
    out_pool = ctx.enter_context(tc.tile_pool(name="out", bufs=8))
    recip_pool = ctx.enter_context(tc.tile_pool(name="recip", bufs=8))
    # PSUM: all 8 banks cycle through one pool so the AV stage can keep
    # all four strips' tiles in flight ahead of their evacuations.
    pp_mm = ctx.enter_context(tc.tile_pool(name="pp_mm", bufs=8, space="PSUM"))

    # PE warmup spin: keep the tensor engine busy from t~0 (no DMA deps)
    # so the HAM clock gate releases to 2.4 GHz before real work arrives.
    warm = consts.tile([P, 2 * P], PD)
    nc.vector.memset(warm[:], 1.0)
    wpsum = pp_mm.tile([P, 2 * P], F32, tag="mm")
    for _ in range(WARMUP_MM):
        nc.tensor.matmul(wpsum[:], warm[:, 0:P], warm[:], start=True, stop=True)

    # Startup critical path: nothing can transfer until the ~7us engine
    # preamble finishes, and HWDGE packets from all queues share the same
    # 16 DMA engines (~220GB/s aggregate) — so what matters is PRIORITY
    # order, not queue parallelism. The head-0 x block rides gpsimd while
    # all weights go on the sync queue in consumption order (q, k, v, out):
    # q+x land first (~12us) so the q-projection can start, and each later
    # block arrives before the stage that needs it.
    wq = consts.tile([P, KC, C], PD)
    wk = consts.tile([P, KC, C], PD)
    wv = consts.tile([P, KC, C], PD)
    wout = consts.tile([P, KC, C], PD)
    xth0 = xt_pool.tile([P, KC, LH], PD, tag="xth")
    nc.gpsimd.dma_start(xth0[:], xh[0])
    nc.sync.dma_start(wq[:], wqh[:])
    nc.sync.dma_start(wk[:], wkh[:])
    nc.sync.dma_start(wv[:], wvh[:])
    nc.sync.dma_start(wout[:], woh[:])

    for h in range(HEADS):
        if h == 0:
            xth = xth0
        else:
            xth = xt_pool.tile([P, KC, LH], PD, tag="xth")
            # single-engine background transfer; the gpsimd rail carries
            # ONLY x prefetches so this trigger is never queued behind a
            # dependent DMA and fires as soon as the buffer frees
            nc.gpsimd.dma_start(xth[:], xh[h])

        # ---- projections Q,K natural (l, c); q evacuates via the scalar
        # engine, k via vector, so neither engine backlogs the scores ----
        q = q_pool.tile([P, KC, C], PD)
        k = k_pool.tile([P, KC, C], PD)
        COPY_FN = mybir.ActivationFunctionType.Copy
        for w_j, dst in ((wq, q), (wk, k)):  # all q strips, then all k:
            for m in range(KC):  # head 0 can stream before wk has landed
                pq = pp_mm.tile([P, C], F32, tag="mm")
                for ko in range(KC):
                    nc.tensor.matmul(
                        pq[:], xth[:, ko, bass.ts(m, P)],
                        w_j[:, ko, :],
                        start=(ko == 0), stop=(ko == KC - 1))
                if dst is q:
                    nc.scalar.activation(dst[:, m, :], pq[:], COPY_FN)
                else:
                    nc.vector.tensor_copy(dst[:, m, :], pq[:])

        # ---- V^T projection sits between the q/k projections and scores:
        # its operands are ready early, and it gives the PE ~3.4us of work
        # while the last q/k strips evacuate ahead of the scores stage ----
        vt = vt_pool.tile([P, KC, LH + 2], PD)
        nc.vector.memset(vt[:, :, 0:2], 1.0)
        for m in range(KC):  # c_v strips of 128
            pv = pp_mm.tile([P, LH], F32, tag="mm")
            for ko in range(KC):
                nc.tensor.matmul(
                    pv[:], wv[:, ko, bass.ds(m * P, P)],
                    xth[:, ko, :],
                    start=(ko == 0), stop=(ko == KC - 1))
            nc.vector.tensor_copy(vt[:, m, 2:LH + 2], pv[:])

        # ---- scores transposed + exp:  S^T[d, c] = sum_l K[l,d] Q[l,c] ----
        ex = exp_pool.tile([P, KC, C], PD)
        for ds_ in range(KC):  # d strips of 128
            ps = pp_mm.tile([P, C], F32, tag="mm")
            for m in range(KC):  # contraction over l' chunks
                nc.tensor.matmul(
                    ps[:], k[:, m, bass.ts(ds_, P)],
                    q[:, m, :],
                    start=(m == 0), stop=(m == KC - 1))
            nc.scalar.activation(ex[:, ds_, :], ps[:], EXP)

        # ---- AV with fused denominator (rhs cols 0,1 are ones; the even
        # N-split respects the 512-fp32 PSUM bank limit). Normalize is a
        # scalar-engine Copy with per-partition scale 1/denominator. ----
        NY1 = 258  # 2 (denominator twice) + 256 v columns
        NY2 = 256
        y = y_pool.tile([P, KC, LH], PD)
        for cs in range(KC):  # c strips of 128
            py1 = pp_mm.tile([P, NY1], F32, tag="mm")
            py2 = pp_mm.tile([P, NY2], F32, tag="mm")
            for ko in range(KC):  # contraction over d chunks
                lhsT = ex[:, ko, bass.ts(cs, P)]
                nc.tensor.matmul(py1[:], lhsT, vt[:, ko, 0:NY1],
                                 start=(ko == 0), stop=(ko == KC - 1))
            for ko in range(KC):
                lhsT = ex[:, ko, bass.ts(cs, P)]
                nc.tensor.matmul(py2[:], lhsT, vt[:, ko, NY1:LH + 2],
                                 start=(ko == 0), stop=(ko == KC - 1))
            rc = recip_pool.tile([P, 1], F32)
            nc.vector.reciprocal(rc[:], py1[:, 0:1])
            nc.scalar.activation(y[:, cs, 0:NY1 - 2], py1[:, 2:NY1], COPY_FN,
                                 scale=rc[:])
            nc.vector.tensor_scalar_mul(y[:, cs, NY1 - 2:LH], py2[:], rc[:])

        # ---- out projection: out[l, co] = sum_c y^T[c, l] woutT[c, co];
        # strip copies alternate vector/scalar, strip-pair DMAs alternate
        # sync/scalar so the gpsimd rail stays x-only ----
        for t in range(KC // 2):  # pairs of l' strips -> one DMA each
            ot = out_pool.tile([P, 2, C], F32)
            for u in range(2):
                m = 2 * t + u
                po = pp_mm.tile([P, C], F32, tag="mm")
                for ko in range(KC):
                    nc.tensor.matmul(
                        po[:], y[:, ko, bass.ts(m, P)],
                        wout[:, ko, :],
                        start=(ko == 0), stop=(ko == KC - 1))
                if u == 0:
                    nc.vector.tensor_copy(ot[:, u, :], po[:])
                else:
                    nc.scalar.activation(ot[:, u, :], po[:], COPY_FN)
            dst = out_r[:, bass.ds(h * KC + 2 * t, 2), :]
            if t == 0:
                nc.sync.dma_start(dst, ot[:])
            else:
                nc.scalar.dma_start(dst, ot[:])


def _build_program():
    nc = bass.Bass(trn_type="TRN2", target_bir_lowering=False, debug=False,
                   num_devices=N_CORES)
    xh = nc.dram_tensor("xh", [HEADS, P, KC, LH], PD, kind="ExternalInput").ap()
    wqh = nc.dram_tensor("wqh", [P, KC, C], PD, kind="ExternalInput").ap()
    wkh = nc.dram_tensor("wkh", [P, KC, C], PD, kind="ExternalInput").ap()
    wvh = nc.dram_tensor("wvh", [P, KC, C], PD, kind="ExternalInput").ap()
    woh = nc.dram_tensor("woh", [P, KC, C], PD, kind="ExternalInput").ap()
    out = nc.dram_tensor("out", [L, C], F32, kind="ExternalOutput").ap()

    from contextlib import ExitStack
    with tile.TileContext(nc) as tc:
        with ExitStack() as ctx:
            _emit(ctx, tc, xh, wqh, wkh, wvh, woh, out)
    _split_wide_waits(nc)
    return nc


def _w_host(w_t):
    """(C, N) transposed weight -> (P, KC, N): row c = ko*P + p goes to
    [p, ko, :], contiguous per partition so the DMA uses large segments."""
    n = w_t.shape[1]
    return np.ascontiguousarray(w_t.reshape(KC, P, n).transpose(1, 0, 2))


def _host_inputs(x, w_qkv, w_out):
    """Per-core input maps, all bf16, pre-permuted so on-device DMAs are
    contiguous per partition. l is permuted so head h owns rows
    [h*512, (h+1)*512) (original row i*8+h -> permuted row h*512+i), and
    x ships transposed (c on partitions)."""
    import ml_dtypes
    BF16 = ml_dtypes.bfloat16
    wqkv_t = np.ascontiguousarray(w_qkv.T).astype(np.float32).copy()
    wqkv_t[:, 0:C] *= SCALE  # fold the attention scale into the Q weights
    wqkv_t = wqkv_t.astype(BF16)
    wqh = _w_host(wqkv_t[:, 0:C])
    wkh = _w_host(wqkv_t[:, C:2 * C])
    wvh = _w_host(wqkv_t[:, 2 * C:])
    woh = _w_host(np.ascontiguousarray(w_out.T).astype(BF16))
    in_maps = []
    for b in range(B):
        xb = x[b]  # (L, C); row l = i*8 + h
        x_perm = xb.reshape(LH, HEADS, C).transpose(1, 0, 2).reshape(L, C)
        xt = np.ascontiguousarray(x_perm.T).astype(BF16)  # (C, L)
        # (HEADS, P, KC, LH): xh[h, p, ko, l] = xt[ko*P + p, h*LH + l]
        xh = np.ascontiguousarray(
            xt.reshape(KC, P, HEADS, LH).transpose(2, 1, 0, 3))
        in_maps.append({"xh": xh, "wqh": wqh, "wkh": wkh, "wvh": wvh,
                        "woh": woh})
    return in_maps


def _unpermute(out_perm):
    """(L, C) with rows grouped by head -> original row order i*8+h."""
    return out_perm.reshape(HEADS, LH, C).transpose(1, 0, 2).reshape(L, C)


def kernel(x, w_qkv, w_out, b_out, _run_kwargs=None):
    x = np.asarray(x, dtype=np.float32)
    w_qkv = np.asarray(w_qkv, dtype=np.float32)
    w_out = np.asarray(w_out, dtype=np.float32)
    b_out = np.asarray(b_out, dtype=np.float32)

    nc = _build_program()
    in_maps = _host_inputs(x, w_qkv, w_out)
    res = run_bass_kernel_spmd(nc, in_maps, list(range(N_CORES)),
                               **(_run_kwargs or {}))
    out = np.empty((B, L, C), dtype=np.float32)
    for b in range(B):
        out[b] = _unpermute(res.results[b]["out"])
    out += b_out
    if _run_kwargs:
        kernel.last_result = res
    return out

